# revision 1
# baseline (speedup 1.0000x reference)
"""MoE (GPT MLP, top-2, GShard capacity) kernel for 8 Trainium2 NeuronCores.

v3: bf16 matmuls, SBUF-resident expert weights, bf16 output, single-DMA
x loads.

Strategy (expert-parallel, matching the sharding hint):
  - Host: fp32 gate (softmax + top-2 + GShard capacity positions), dispatch
    gather.  Routing is O(N*E) int/scalar work - negligible next to the FFN -
    and the capacity scan is inherently sequential, so it runs on host.
  - Device: 8 cores, core e owns expert e.  Each core runs the expert FFN
    y = gelu(disp @ w1 + b1) @ w2 over its cap=2048 dispatched token slots.
    Matmuls in bf16 (fp32 PSUM accumulate): same PE rate as fp32r but the
    weights (w1+w2 = 16.8 MB in bf16) fit in SBUF, so they are loaded once
    and never re-streamed; x streams per 512-token group.  PSUM chains are
    one bank each with disjoint rings for the two phases, so the PE never
    idles at phase boundaries.
  - Host: combine (gather + gate-weighted sum) + b2.

Self-contained: hardcodes B=4, S=2048, D=1024, H=4096, E=8, K=2, cap=2048.
"""

import sys

sys.path.insert(0, "/opt/trn_rl_repo")

import numpy as np
import ml_dtypes

B, S, D, H, E = 4, 2048, 1024, 4096, 8
K = 2
N_TOK = B * S            # 8192
CAP = (K * N_TOK) // E   # 2048 (capacity factor 1.0)
EPS = 1e-9
P = 128                  # SBUF partitions

BF16 = ml_dtypes.bfloat16

_NC_CACHE = {}


# --------------------------------------------------------------------------
# Host routing (replicates reference.py's gate exactly, in numpy fp32)
# --------------------------------------------------------------------------

def _route(xt, wg):
    """xt: [N, D] fp32, wg: [D, E] fp32 ->
    gidx [N,K] int, gvals [N,K] fp32 (keep-masked), pos [N,K] int, keep [N,K]"""
    logits = xt @ wg                                   # [N, E] fp32
    m = logits.max(axis=-1, keepdims=True)
    ex = np.exp(logits - m)
    scores = ex / ex.sum(axis=-1, keepdims=True)
    order = np.argsort(-scores, axis=1, kind="stable")  # jax top_k tie rule
    gidx = order[:, :K]                                 # [N, K]
    gvals = np.take_along_axis(scores, gidx, axis=1)
    gvals = gvals / np.clip(gvals.sum(-1, keepdims=True), EPS, None)

    n = xt.shape[0]
    offset = np.zeros(E, np.int64)
    pos = np.zeros((n, K), np.int64)
    keep = np.zeros((n, K), bool)
    rows = np.arange(n)
    for kk in range(K):
        ek = gidx[:, kk]
        oh = np.zeros((n, E), np.int64)
        oh[rows, ek] = 1
        loc = np.cumsum(oh, axis=0) - 1 + offset[None, :]
        offset = offset + oh.sum(axis=0)
        p = loc[rows, ek]
        kmask = p < CAP
        pos[:, kk] = np.where(kmask, p, 0)
        keep[:, kk] = kmask
    gvals = (gvals * keep).astype(np.float32)
    return gidx, gvals, pos, keep


# --------------------------------------------------------------------------
# Device kernel builder (one expert FFN per core, SPMD)
# --------------------------------------------------------------------------

def _build_nc(d, h, ntok, debug=False, act="Gelu", reps=1, loop_trip=None):
    """Expert FFN: y[ntok, d] = gelu(x[ntok, d] @ w1[d, h] + b1[h]) @ w2[h, d].

    All matmul operands bf16; w1/w2 resident in SBUF for the whole program.
    Per 512-token group: phase A fills h[P, mt_n, 512] (32 chains of 8
    accumulating matmuls, one PSUM bank each, gelu drain to bf16 SBUF);
    phase B contracts h against resident w2 (8 chains of 32 matmuls, one
    bank each, DVE drain + DMA out).  Issue order per rep:
       m1(0); m2(0); m1(1); m2(1); ...
    Since the PE runs in order, every instruction's deps are complete when
    it reaches the head: no PE stalls except ring-buffer waits (slack >=
    2 chains).

    Device inputs:
      xb  : [P, ntok/tgs, d/P, tgs] bf16  x[j*P+p, g*tgs+s] at [p, g, j, s]
                                          (one contiguous DMA per group)
      w1b : [P, d/P, h/P, P] bf16  w1[j*P+p, m*P+c] at [p, j, m, c]
      w2b : [P, h/P, d] bf16       w2[m*P+p, :] at [p, m, :]
      b1t : [P, h/P] fp32          b1 transposed
    Output:
      y   : [ntok/P, P, d] bf16
    """
    import contextlib

    from concourse import bacc, mybir, tile

    dt_n = d // P            # 8   D tiles (contraction tiles for matmul1)
    mt_n = h // P            # 32  H tiles
    tgs = min(512, ntok)     # tokens per group
    tg_n = ntok // tgs       # 4
    tt_n = tgs // P          # 4   128-token tiles per group
    dh_n = (d + 511) // 512  # 2   output D splits (PSUM bank = 512 fp32)

    f32 = mybir.dt.float32
    bf = mybir.dt.bfloat16
    actf = getattr(mybir.ActivationFunctionType, act)

    nc = bacc.Bacc("TRN2", target_bir_lowering=False, debug=debug,
                   enable_asserts=False, num_devices=1)

    xb_d = nc.dram_tensor("xb", [P, tg_n, dt_n, tgs], bf, kind="ExternalInput")
    w1_d = nc.dram_tensor("w1b", [P, dt_n, mt_n, P], bf, kind="ExternalInput")
    w2_d = nc.dram_tensor("w2b", [P, mt_n, d], bf, kind="ExternalInput")
    b1_d = nc.dram_tensor("b1t", [P, mt_n], f32, kind="ExternalInput")
    y_d = nc.dram_tensor("y", [ntok // P, P, d], bf, kind="ExternalOutput")

    with tile.TileContext(nc) as tc:
        with (
            tc.tile_pool(name="cpool", bufs=1) as cpool,
            tc.tile_pool(name="xgpool", bufs=2) as xgpool,
            tc.tile_pool(name="hpool", bufs=1) as hpool,
            tc.tile_pool(name="ypool", bufs=4) as ypool,
            tc.tile_pool(name="ps1", bufs=4, space="PSUM") as ps1pool,
            tc.tile_pool(name="ps2", bufs=4, space="PSUM") as ps2pool,
        ):
            b1_t = cpool.tile([P, mt_n], f32, name="b1_t")
            w1_t = cpool.tile([P, dt_n, mt_n, P], bf, name="w1_t")
            w2_t = cpool.tile([P, mt_n, d], bf, name="w2_t")
            nc.sync.dma_start(b1_t[:], b1_d[:])
            nc.sync.dma_start(w1_t[:], w1_d[:])
            nc.sync.dma_start(w2_t[:], w2_d[:])

            # loop_trip: hardware For_i loop around the rep body (used for
            # low-variance timing: small program, long execution)
            loop_cm = (tc.For_i(0, loop_trip) if loop_trip
                       else contextlib.nullcontext())
            with loop_cm:
              for rep in range(reps):
                hs = []   # h tile per group (pool bufs=1 -> same buffer)
                for tg in range(tg_n):
                    t0, t1 = tg * tgs, (tg + 1) * tgs
                    # ---- phase A: h = gelu(x @ w1 + b1) for this group ----
                    xg = xgpool.tile([P, dt_n, tgs], bf, tag="xg",
                                     name=f"xg_{rep}_{tg}")
                    nc.sync.dma_start(xg[:], xb_d[:, tg])
                    ht = hpool.tile([P, mt_n, tgs], bf, tag="h",
                                    name=f"h_{rep}_{tg}")
                    for m in range(mt_n):
                        ps = ps1pool.tile([P, tgs], f32, tag="ps1",
                                          name=f"ps1_{rep}_{tg}_{m}")
                        for j in range(dt_n):
                            nc.tensor.matmul(
                                ps[:],
                                w1_t[:, j, m, :],
                                xg[:, j, :],
                                start=(j == 0),
                                stop=(j == dt_n - 1),
                            )
                        nc.scalar.activation(
                            ht[:, m, :], ps[:], actf,
                            bias=b1_t[:, m:m + 1], scale=1.0,
                        )
                    # ---- phase B: y = h @ w2 for this group ----
                    for t in range(tt_n):
                        c0 = t * P
                        for dh in range(dh_n):
                            d0, d1 = dh * 512, min((dh + 1) * 512, d)
                            ps = ps2pool.tile([P, d1 - d0], f32, tag="ps2",
                                              name=f"ps2_{rep}_{tg}_{t}_{dh}")
                            for m in range(mt_n):
                                nc.tensor.matmul(
                                    ps[:],
                                    ht[:, m, c0:c0 + P],
                                    w2_t[:, m, d0:d1],
                                    start=(m == 0),
                                    stop=(m == mt_n - 1),
                                )
                            yt = ypool.tile([P, d1 - d0], bf, tag="yt",
                                            name=f"yt_{rep}_{tg}_{t}_{dh}")
                            nc.vector.tensor_copy(yt[:], ps[:])
                            nc.sync.dma_start(
                                y_d[tg * tt_n + t][:, d0:d1], yt[:])

    nc.compile()
    return nc


def _get_nc(d, h, ntok, debug=False, reps=1, loop_trip=None):
    key = (d, h, ntok, debug, reps, loop_trip)
    if key not in _NC_CACHE:
        _NC_CACHE[key] = _build_nc(d, h, ntok, debug, reps=reps,
                                   loop_trip=loop_trip)
    return _NC_CACHE[key]


# --------------------------------------------------------------------------
# Host-side input layout per core
# --------------------------------------------------------------------------

def _core_inputs(disp_e, w1_e, w2_e, b1_e):
    """disp_e: [CAP, D], w1_e: [D, H], w2_e: [H, D], b1_e: [H]."""
    tgs = min(512, CAP)
    xb = np.ascontiguousarray(
        disp_e.T.astype(BF16).reshape(D // P, P, CAP // tgs, tgs)
        .transpose(1, 2, 0, 3))
    w1b = np.ascontiguousarray(
        w1_e.astype(BF16).reshape(D // P, P, H // P, P).transpose(1, 0, 2, 3))
    w2b = np.ascontiguousarray(
        w2_e.astype(BF16).reshape(H // P, P, D).transpose(1, 0, 2))
    b1t = np.ascontiguousarray(b1_e.reshape(H // P, P).T)
    return {"xb": xb, "w1b": w1b, "w2b": w2b, "b1t": b1t}


def _get_runner(nc, n_cores):
    """Cached PJRT executable for an SPMD bass program (axon path of
    run_bass_kernel_spmd, with the jitted callable kept warm across calls)."""
    key = id(nc)
    if key in _NC_CACHE:
        return _NC_CACHE[key]

    import jax
    from jax.sharding import Mesh, PartitionSpec
    from jax.experimental.shard_map import shard_map
    from concourse import mybir
    from concourse.bass2jax import (_bass_exec_p, install_neuronx_cc_hook,
                                    partition_id_tensor)

    install_neuronx_cc_hook()

    partition_name = (nc.partition_id_tensor.name
                      if nc.partition_id_tensor else None)
    in_names, out_names, out_avals = [], [], []
    for alloc in nc.m.functions[0].allocations:
        if not isinstance(alloc, mybir.MemoryLocationSet):
            continue
        name = alloc.memorylocations[0].name
        if alloc.kind == "ExternalInput":
            if name != partition_name:
                in_names.append(name)
        elif alloc.kind == "ExternalOutput":
            out_names.append(name)
            shape = tuple(alloc.tensor_shape)
            out_avals.append(jax.core.ShapedArray(shape, mybir.dt.np(alloc.dtype)))
    n_params = len(in_names)
    n_outs = len(out_avals)
    in_names = in_names + out_names
    if partition_name is not None:
        in_names.append(partition_name)
    donate = tuple(range(n_params, n_params + n_outs))

    def _body(*args):
        operands = list(args)
        if partition_name is not None:
            operands.append(partition_id_tensor())
        outs = _bass_exec_p.bind(
            *operands,
            out_avals=tuple(out_avals),
            in_names=tuple(in_names),
            out_names=tuple(out_names),
            lowering_input_output_aliases=(),
            sim_require_finite=True,
            sim_require_nnan=True,
            nc=nc,
        )
        return tuple(outs)

    devices = jax.devices()[:n_cores]
    mesh = Mesh(np.asarray(devices), ("core",))
    in_specs = (PartitionSpec("core"),) * (n_params + n_outs)
    out_specs = (PartitionSpec("core"),) * n_outs
    sharded = jax.jit(
        shard_map(_body, mesh=mesh, in_specs=in_specs, out_specs=out_specs,
                  check_rep=False),
        donate_argnums=donate, keep_unused=True,
    )

    def run(in_maps, reps=1, time_reps=False):
        import time as _time
        concat_in = [
            np.concatenate([np.asarray(m[in_names[i]]) for m in in_maps], axis=0)
            for i in range(n_params)
        ]
        concat_in = [jax.device_put(a) for a in concat_in]
        zero_sets = []
        for _ in range(reps):
            zero_sets.append([
                jax.device_put(np.zeros((n_cores * av.shape[0], *av.shape[1:]),
                                        av.dtype))
                for av in out_avals
            ])
        for zs in zero_sets:
            for z in zs:
                z.block_until_ready()
        for a in concat_in:
            a.block_until_ready()
        times = []
        out_arrs = None
        for r in range(reps):
            t0 = _time.perf_counter()
            out_arrs = sharded(*concat_in, *zero_sets[r])
            for o in out_arrs:
                o.block_until_ready()
            times.append(_time.perf_counter() - t0)
        results = [
            {name: np.asarray(out_arrs[i]).reshape(n_cores, *out_avals[i].shape)[c]
             for i, name in enumerate(out_names)}
            for c in range(n_cores)
        ]
        if time_reps:
            return results, times
        return results

    _NC_CACHE[key] = run
    return run


def _make_in_maps(x, wg, w1, b1, w2):
    xt = x.reshape(N_TOK, D)
    gidx, gvals, pos, keep = _route(xt, wg)
    disp = np.zeros((E, CAP, D), np.float32)
    for kk in range(K):
        tok = np.nonzero(keep[:, kk])[0]
        disp[gidx[tok, kk], pos[tok, kk]] = xt[tok]
    in_maps = [_core_inputs(disp[e], w1[e], w2[e], b1[e]) for e in range(E)]
    return in_maps, gidx, gvals, pos


def kernel(x, wg, w1, b1, w2, b2):
    x = np.asarray(x, np.float32)
    wg = np.asarray(wg, np.float32)
    w1 = np.asarray(w1, np.float32)
    b1 = np.asarray(b1, np.float32)
    w2 = np.asarray(w2, np.float32)
    b2 = np.asarray(b2, np.float32)

    in_maps, gidx, gvals, pos = _make_in_maps(x, wg, w1, b1, w2)

    nc = _get_nc(D, H, CAP)
    run = _get_runner(nc, E)
    results = run(in_maps)
    y_all = np.stack([r["y"].astype(np.float32).reshape(CAP, D)
                      for r in results])  # [E,CAP,D]

    # combine: out = sum_k gvals * (y[e, pos] + b2[e])
    e_flat = gidx.reshape(-1)
    p_flat = pos.reshape(-1)
    yk = y_all[e_flat, p_flat] + b2[e_flat]
    w = gvals.reshape(-1).astype(np.float32)
    out = (yk * w[:, None]).reshape(N_TOK, K, D).sum(axis=1)
    return out.reshape(B, S, D).astype(np.float32)


# --------------------------------------------------------------------------
# Benchmarking helpers (test.py only)
# --------------------------------------------------------------------------

def bench(x, wg, w1, b1, w2, b2, reps=10, rep_counts=(1, 5)):
    """Returns per-call wall-second lists for each rep_count kernel.

    Calls of the different rep_count executables are interleaved so that
    thermal / tunnel conditions are sampled identically for both."""
    x = np.asarray(x, np.float32)
    in_maps, _, _, _ = _make_in_maps(
        x, np.asarray(wg, np.float32), np.asarray(w1, np.float32),
        np.asarray(b1, np.float32), np.asarray(w2, np.float32))
    runs = []
    for rc in rep_counts:
        nc = _get_nc(D, H, CAP, reps=rc)
        runs.append(_get_runner(nc, E))
    for run in runs:           # warm both executables (compile/transfer)
        run(in_maps, reps=2)
    out = [[] for _ in rep_counts]
    for _ in range(reps):
        for i, run in enumerate(runs):
            _, t = run(in_maps, reps=1, time_reps=True)
            out[i].append(t[0])
    return out


def bench_loop(x, wg, w1, b1, w2, b2, trips=(4, 504), calls=6, body_reps=2):
    """Per-pass device time via hardware-loop (For_i) trip-count slope.

    The program body is one full FFN pass (x DMA in, matmul1, gelu,
    matmul2, y DMA out); the loop repeats it trip times on-device.  The
    wall-time difference between trip counts divides out per-call host and
    tunnel overhead (~100 ms, +-10 ms) over hundreds of passes, giving a
    low-variance per-pass estimate.  Weights stay SBUF-resident across
    passes, exactly as in kernel().

    Returns (per_pass_seconds, {trip: [wall_times]}).
    """
    x = np.asarray(x, np.float32)
    in_maps, _, _, _ = _make_in_maps(
        x, np.asarray(wg, np.float32), np.asarray(w1, np.float32),
        np.asarray(b1, np.float32), np.asarray(w2, np.float32))
    walls = {}
    for trip in trips:
        nc = _get_nc(D, H, CAP, reps=body_reps, loop_trip=trip)
        run = _get_runner(nc, E)
        _, t = run(in_maps, reps=calls, time_reps=True)
        walls[trip] = t
    lo = min(walls[trips[0]][1:])
    hi = min(walls[trips[1]][1:])
    per_pass = (hi - lo) / ((trips[1] - trips[0]) * body_reps)
    return per_pass, walls



# revision 6
# speedup vs baseline: 1.2166x; 1.2166x over previous
"""MoE (GPT MLP, top-2, GShard capacity) kernel for 8 Trainium2 NeuronCores.

v4: compound matmuls (2048-row moving operand, 4 PSUM banks) so each
128x128 stationary weight block is loaded exactly once per pass; weights
stream from HBM (overlapped) instead of staying SBUF-resident.

Strategy (expert-parallel, matching the sharding hint):
  - Host: fp32 gate (softmax + top-2 + GShard capacity positions), dispatch
    gather.  Routing is O(N*E) int/scalar work - negligible next to the FFN -
    and the capacity scan is inherently sequential, so it runs on host.
  - Device: 8 cores, core e owns expert e.  Each core runs the expert FFN
    y = gelu(disp @ w1 + b1) @ w2 over its cap=2048 dispatched token slots.
    Matmuls in bf16 (fp32 PSUM accumulate).  Phase A: per H-tile m, one
    8-matmul chain accumulates over D-tiles with all 2048 tokens moving
    per instruction (out [128, 2048] = 4 PSUM banks); gelu+bias drains to
    bf16 h in SBUF.  Phase B: per D-block, one 32-matmul chain contracts
    h against w2 blocks, again 2048 tokens moving.  256+256 matmul
    instructions per pass, one LDWEIGHTS per weight block (the minimum).
    w1/w2 stream from HBM per-use (24 MB/pass, ~70 us at HBM rate) fully
    under the ~460 us of PE work; x and h stay SBUF-resident.
  - Host: combine (gather + gate-weighted sum) + b2.

Self-contained: hardcodes B=4, S=2048, D=1024, H=4096, E=8, K=2, cap=2048.
"""

import sys

sys.path.insert(0, "/opt/trn_rl_repo")

import numpy as np
import ml_dtypes

B, S, D, H, E = 4, 2048, 1024, 4096, 8
K = 2
N_TOK = B * S            # 8192
CAP = (K * N_TOK) // E   # 2048 (capacity factor 1.0)
EPS = 1e-9
P = 128                  # SBUF partitions

BF16 = ml_dtypes.bfloat16

_NC_CACHE = {}


# --------------------------------------------------------------------------
# Host routing (replicates reference.py's gate exactly, in numpy fp32)
# --------------------------------------------------------------------------

def _route(xt, wg):
    """xt: [N, D] fp32, wg: [D, E] fp32 ->
    gidx [N,K] int, gvals [N,K] fp32 (keep-masked), pos [N,K] int, keep [N,K]"""
    logits = xt @ wg                                   # [N, E] fp32
    m = logits.max(axis=-1, keepdims=True)
    ex = np.exp(logits - m)
    scores = ex / ex.sum(axis=-1, keepdims=True)
    order = np.argsort(-scores, axis=1, kind="stable")  # jax top_k tie rule
    gidx = order[:, :K]                                 # [N, K]
    gvals = np.take_along_axis(scores, gidx, axis=1)
    gvals = gvals / np.clip(gvals.sum(-1, keepdims=True), EPS, None)

    n = xt.shape[0]
    offset = np.zeros(E, np.int64)
    pos = np.zeros((n, K), np.int64)
    keep = np.zeros((n, K), bool)
    rows = np.arange(n)
    for kk in range(K):
        ek = gidx[:, kk]
        oh = np.zeros((n, E), np.int64)
        oh[rows, ek] = 1
        loc = np.cumsum(oh, axis=0) - 1 + offset[None, :]
        offset = offset + oh.sum(axis=0)
        p = loc[rows, ek]
        kmask = p < CAP
        pos[:, kk] = np.where(kmask, p, 0)
        keep[:, kk] = kmask
    gvals = (gvals * keep).astype(np.float32)
    return gidx, gvals, pos, keep


# --------------------------------------------------------------------------
# Device kernel builder (one expert FFN per core, SPMD)
# --------------------------------------------------------------------------

def _build_nc(d, h, ntok, debug=False, act="Gelu", reps=1, loop_trip=None):
    """Expert FFN: y[ntok, d] = gelu(x[ntok, d] @ w1[d, h] + b1[h]) @ w2[h, d].

    Compound matmuls: every matmul streams all ntok tokens (out
    [128, ntok] fp32 = ntok/512 PSUM banks), so each 128x128 stationary
    block needs exactly one LDWEIGHTS.  w1 streams per H-tile, w2 per
    D-block (double/triple-buffered); x and h stay resident.

    Device inputs:
      xb  : [d/P, P, ntok] bf16     x[t, j*P+p] at [j, p, t]
      w1b : [h/P, P, d/P, P] bf16   w1[j*P+p, m*P+c] at [m, p, j, c]
      w2b : [d/P, P, h/P, P] bf16   w2[m*P+p, db*P+c] at [db, p, m, c]
      b1t : [P, h/P] fp32           b1 transposed
    Output:
      y   : [d/P, P, ntok] bf16     y[t, db*P+p] at [db, p, t]
    """
    import contextlib

    from concourse import bacc, mybir, tile

    dt_n = d // P            # 8   D tiles (contraction tiles for matmul1)
    mt_n = h // P            # 32  H tiles
    db_n = d // P            # 8   output D blocks for matmul2
    bk_n = ntok // 512       # 4   PSUM banks per compound matmul
    bks = 512                # moving rows per PSUM bank

    f32 = mybir.dt.float32
    bf = mybir.dt.bfloat16
    actf = getattr(mybir.ActivationFunctionType, act)

    nc = bacc.Bacc("TRN2", target_bir_lowering=False, debug=debug,
                   enable_asserts=False, num_devices=1)

    xb_d = nc.dram_tensor("xb", [dt_n, P, ntok], bf, kind="ExternalInput")
    w1_d = nc.dram_tensor("w1b", [mt_n, P, dt_n, P], bf, kind="ExternalInput")
    w2_d = nc.dram_tensor("w2b", [db_n, P, mt_n, P], bf, kind="ExternalInput")
    b1_d = nc.dram_tensor("b1t", [P, mt_n], f32, kind="ExternalInput")
    y_d = nc.dram_tensor("y", [db_n, P, ntok], bf, kind="ExternalOutput")

    with tile.TileContext(nc) as tc:
        with (
            tc.tile_pool(name="cpool", bufs=1) as cpool,
            tc.tile_pool(name="w1pool", bufs=3) as w1pool,
            tc.tile_pool(name="w2pool", bufs=2) as w2pool,
            tc.tile_pool(name="ypool", bufs=2) as ypool,
            tc.tile_pool(name="ps", bufs=2, space="PSUM") as pspool,
        ):
            b1_t = cpool.tile([P, mt_n], f32, name="b1_t")
            x_t = cpool.tile([P, dt_n, bk_n, bks], bf, name="x_t")
            h_t = cpool.tile([P, mt_n, bk_n, bks], bf, name="h_t")
            nc.sync.dma_start(b1_t[:], b1_d[:])

            # loop_trip: hardware For_i loop around the rep body (used for
            # low-variance timing: small program, long execution)
            loop_cm = (tc.For_i(0, loop_trip) if loop_trip
                       else contextlib.nullcontext())
            with loop_cm:
              for rep in range(reps):
                for j in range(dt_n):
                    nc.sync.dma_start(x_t[:, j], xb_d[j])
                # ---- phase A: h = gelu(x @ w1 + b1), all tokens ----
                for m in range(mt_n):
                    w1t = w1pool.tile([P, dt_n, P], bf, tag="w1",
                                      name=f"w1_{rep}_{m}")
                    nc.sync.dma_start(w1t[:], w1_d[m])
                    ps = pspool.tile([P, bk_n, bks], f32, tag="ps",
                                     name=f"ps1_{rep}_{m}")
                    for j in range(dt_n):
                        for k in range(bk_n):
                            nc.tensor.matmul(
                                ps[:, k],
                                w1t[:, j, :],
                                x_t[:, j, k],
                                start=(j == 0),
                                stop=(j == dt_n - 1),
                            )
                    nc.scalar.activation(
                        h_t[:, m], ps[:], actf,
                        bias=b1_t[:, m:m + 1], scale=1.0,
                    )
                # ---- phase B: y = h @ w2, all tokens ----
                for db in range(db_n):
                    w2t = w2pool.tile([P, mt_n, P], bf, tag="w2",
                                      name=f"w2_{rep}_{db}")
                    nc.sync.dma_start(w2t[:], w2_d[db])
                    ps = pspool.tile([P, bk_n, bks], f32, tag="ps",
                                     name=f"ps2_{rep}_{db}")
                    for m in range(mt_n):
                        for k in range(bk_n):
                            nc.tensor.matmul(
                                ps[:, k],
                                w2t[:, m, :],
                                h_t[:, m, k],
                                start=(m == 0),
                                stop=(m == mt_n - 1),
                            )
                    yt = ypool.tile([P, ntok], bf, tag="yt",
                                    name=f"yt_{rep}_{db}")
                    nc.vector.tensor_copy(yt[:], ps[:])
                    nc.sync.dma_start(y_d[db], yt[:])

    nc.compile()
    _dedup_ldweights(nc)
    return nc


def _ldw_key(inst):
    """Identity key for an InstLdweights: the physical weights AP plus the
    load-mode flags.  Two consecutive LDWs with equal keys load identical
    PE-array contents (same SBUF address/shape; no DMA can rewrite that
    address between them without an intervening differently-keyed LDW in
    this kernel's emission order, because weight-pool buffers are only
    recycled after other chains' LDWs)."""
    return (str(inst.ins[0]), inst.perf_mode, inst.is_transpose,
            str(inst.tile_position), str(inst.tile_size))


def _dedup_ldweights(nc):
    """Remove redundant PE weight loads after compilation.

    tile_legalize splits every bf16 matmul into InstLdweights +
    InstMatmult(ldweights=False).  Matmuls emitted back-to-back against
    the same stationary block (the bk_n per-bank copies of one compound
    matmul) therefore reload the PE array bk_n times; all but the first
    load are no-ops.  Drop an InstLdweights when (a) the previous
    PE-queue instruction sequence since the last kept LDW contains only
    non-self-loading matmuls / event semaphores and that LDW has an
    identical key, and (b) it carries no semaphore waits or updates (the
    chain-leading LDW, which inherits the weight-DMA wait, always
    differs in key from its predecessor and is kept).
    """
    from concourse import mybir

    removed = 0
    for blk in nc.main_func.blocks:
        insts = list(blk.instructions)
        keep = []
        last_key = None
        for inst in insts:
            if isinstance(inst, mybir.InstLdweights):
                si = inst.sync_info
                clean = si is None or (len(si.on_wait) == 0
                                       and len(si.on_update) == 0)
                key = _ldw_key(inst)
                if clean and key == last_key:
                    removed += 1
                    continue
                last_key = key
            elif isinstance(inst, mybir.InstMatmult):
                if inst.ldweights is not False:
                    last_key = None        # self-loading: clobbers array
            elif isinstance(inst, mybir.InstEventSemaphore):
                pass                       # pure sync: array untouched
            elif getattr(inst, "engine", None) == mybir.EngineType.PE:
                last_key = None            # unknown PE op: be conservative
            keep.append(inst)
        if removed and len(keep) != len(insts):
            while len(blk.instructions):
                blk.instructions.pop()
            for inst in keep:
                blk.instructions.append(inst)
    return removed


def _get_nc(d, h, ntok, debug=False, reps=1, loop_trip=None):
    key = (d, h, ntok, debug, reps, loop_trip)
    if key not in _NC_CACHE:
        _NC_CACHE[key] = _build_nc(d, h, ntok, debug, reps=reps,
                                   loop_trip=loop_trip)
    return _NC_CACHE[key]


# --------------------------------------------------------------------------
# Host-side input layout per core
# --------------------------------------------------------------------------

def _core_inputs(disp_e, w1_e, w2_e, b1_e):
    """disp_e: [CAP, D], w1_e: [D, H], w2_e: [H, D], b1_e: [H]."""
    xb = np.ascontiguousarray(
        disp_e.T.astype(BF16).reshape(D // P, P, CAP))
    w1b = np.ascontiguousarray(
        w1_e.astype(BF16).reshape(D // P, P, H // P, P).transpose(2, 1, 0, 3))
    w2b = np.ascontiguousarray(
        w2_e.astype(BF16).reshape(H // P, P, D // P, P).transpose(2, 1, 0, 3))
    b1t = np.ascontiguousarray(b1_e.reshape(H // P, P).T)
    return {"xb": xb, "w1b": w1b, "w2b": w2b, "b1t": b1t}


def _get_runner(nc, n_cores):
    """Cached PJRT executable for an SPMD bass program (axon path of
    run_bass_kernel_spmd, with the jitted callable kept warm across calls)."""
    key = id(nc)
    if key in _NC_CACHE:
        return _NC_CACHE[key]

    import jax
    from jax.sharding import Mesh, PartitionSpec
    from jax.experimental.shard_map import shard_map
    from concourse import mybir
    from concourse.bass2jax import (_bass_exec_p, install_neuronx_cc_hook,
                                    partition_id_tensor)

    install_neuronx_cc_hook()

    partition_name = (nc.partition_id_tensor.name
                      if nc.partition_id_tensor else None)
    in_names, out_names, out_avals = [], [], []
    for alloc in nc.m.functions[0].allocations:
        if not isinstance(alloc, mybir.MemoryLocationSet):
            continue
        name = alloc.memorylocations[0].name
        if alloc.kind == "ExternalInput":
            if name != partition_name:
                in_names.append(name)
        elif alloc.kind == "ExternalOutput":
            out_names.append(name)
            shape = tuple(alloc.tensor_shape)
            out_avals.append(jax.core.ShapedArray(shape, mybir.dt.np(alloc.dtype)))
    n_params = len(in_names)
    n_outs = len(out_avals)
    in_names = in_names + out_names
    if partition_name is not None:
        in_names.append(partition_name)
    donate = tuple(range(n_params, n_params + n_outs))

    def _body(*args):
        operands = list(args)
        if partition_name is not None:
            operands.append(partition_id_tensor())
        outs = _bass_exec_p.bind(
            *operands,
            out_avals=tuple(out_avals),
            in_names=tuple(in_names),
            out_names=tuple(out_names),
            lowering_input_output_aliases=(),
            sim_require_finite=True,
            sim_require_nnan=True,
            nc=nc,
        )
        return tuple(outs)

    devices = jax.devices()[:n_cores]
    mesh = Mesh(np.asarray(devices), ("core",))
    in_specs = (PartitionSpec("core"),) * (n_params + n_outs)
    out_specs = (PartitionSpec("core"),) * n_outs
    sharded = jax.jit(
        shard_map(_body, mesh=mesh, in_specs=in_specs, out_specs=out_specs,
                  check_rep=False),
        donate_argnums=donate, keep_unused=True,
    )

    def run(in_maps, reps=1, time_reps=False):
        import time as _time
        concat_in = [
            np.concatenate([np.asarray(m[in_names[i]]) for m in in_maps], axis=0)
            for i in range(n_params)
        ]
        concat_in = [jax.device_put(a) for a in concat_in]
        zero_sets = []
        for _ in range(reps):
            zero_sets.append([
                jax.device_put(np.zeros((n_cores * av.shape[0], *av.shape[1:]),
                                        av.dtype))
                for av in out_avals
            ])
        for zs in zero_sets:
            for z in zs:
                z.block_until_ready()
        for a in concat_in:
            a.block_until_ready()
        times = []
        out_arrs = None
        for r in range(reps):
            t0 = _time.perf_counter()
            out_arrs = sharded(*concat_in, *zero_sets[r])
            for o in out_arrs:
                o.block_until_ready()
            times.append(_time.perf_counter() - t0)
        results = [
            {name: np.asarray(out_arrs[i]).reshape(n_cores, *out_avals[i].shape)[c]
             for i, name in enumerate(out_names)}
            for c in range(n_cores)
        ]
        if time_reps:
            return results, times
        return results

    _NC_CACHE[key] = run
    return run


def _make_in_maps(x, wg, w1, b1, w2):
    xt = x.reshape(N_TOK, D)
    gidx, gvals, pos, keep = _route(xt, wg)
    disp = np.zeros((E, CAP, D), np.float32)
    for kk in range(K):
        tok = np.nonzero(keep[:, kk])[0]
        disp[gidx[tok, kk], pos[tok, kk]] = xt[tok]
    in_maps = [_core_inputs(disp[e], w1[e], w2[e], b1[e]) for e in range(E)]
    return in_maps, gidx, gvals, pos


def kernel(x, wg, w1, b1, w2, b2):
    x = np.asarray(x, np.float32)
    wg = np.asarray(wg, np.float32)
    w1 = np.asarray(w1, np.float32)
    b1 = np.asarray(b1, np.float32)
    w2 = np.asarray(w2, np.float32)
    b2 = np.asarray(b2, np.float32)

    in_maps, gidx, gvals, pos = _make_in_maps(x, wg, w1, b1, w2)

    nc = _get_nc(D, H, CAP)
    run = _get_runner(nc, E)
    results = run(in_maps)
    # device y is [db, p, tok]; token-major per expert is [tok, db*P+p]
    y_all = np.stack([r["y"].astype(np.float32).transpose(2, 0, 1)
                      .reshape(CAP, D) for r in results])  # [E,CAP,D]

    # combine: out = sum_k gvals * (y[e, pos] + b2[e])
    e_flat = gidx.reshape(-1)
    p_flat = pos.reshape(-1)
    yk = y_all[e_flat, p_flat] + b2[e_flat]
    w = gvals.reshape(-1).astype(np.float32)
    out = (yk * w[:, None]).reshape(N_TOK, K, D).sum(axis=1)
    return out.reshape(B, S, D).astype(np.float32)


# --------------------------------------------------------------------------
# Benchmarking helpers (test.py only)
# --------------------------------------------------------------------------

def bench(x, wg, w1, b1, w2, b2, reps=10, rep_counts=(1, 5)):
    """Returns per-call wall-second lists for each rep_count kernel.

    Calls of the different rep_count executables are interleaved so that
    thermal / tunnel conditions are sampled identically for both."""
    x = np.asarray(x, np.float32)
    in_maps, _, _, _ = _make_in_maps(
        x, np.asarray(wg, np.float32), np.asarray(w1, np.float32),
        np.asarray(b1, np.float32), np.asarray(w2, np.float32))
    runs = []
    for rc in rep_counts:
        nc = _get_nc(D, H, CAP, reps=rc)
        runs.append(_get_runner(nc, E))
    for run in runs:           # warm both executables (compile/transfer)
        run(in_maps, reps=2)
    out = [[] for _ in rep_counts]
    for _ in range(reps):
        for i, run in enumerate(runs):
            _, t = run(in_maps, reps=1, time_reps=True)
            out[i].append(t[0])
    return out


def bench_loop(x, wg, w1, b1, w2, b2, trips=(4, 504), calls=6, body_reps=2):
    """Per-pass device time via hardware-loop (For_i) trip-count slope.

    The program body is one full FFN pass (x/w1/w2 DMA in, matmul1, gelu,
    matmul2, y DMA out); the loop repeats it trip times on-device.  The
    wall-time difference between trip counts divides out per-call host and
    tunnel overhead (~100 ms, +-10 ms) over hundreds of passes, giving a
    low-variance per-pass estimate.  All per-pass DMAs (including weight
    streaming) are inside the loop body, exactly as in kernel().

    Returns (per_pass_seconds, {trip: [wall_times]}).
    """
    x = np.asarray(x, np.float32)
    in_maps, _, _, _ = _make_in_maps(
        x, np.asarray(wg, np.float32), np.asarray(w1, np.float32),
        np.asarray(b1, np.float32), np.asarray(w2, np.float32))
    walls = {}
    for trip in trips:
        nc = _get_nc(D, H, CAP, reps=body_reps, loop_trip=trip)
        run = _get_runner(nc, E)
        _, t = run(in_maps, reps=calls, time_reps=True)
        walls[trip] = t
    lo = min(walls[trips[0]][1:])
    hi = min(walls[trips[1]][1:])
    per_pass = (hi - lo) / ((trips[1] - trips[0]) * body_reps)
    return per_pass, walls
